# revision 10
# baseline (speedup 1.0000x reference)
"""Trainium2 Bass kernel for AttentionDecoder (B=48,T=1024,D=512,H=512,F=256,
C=4367,S=22). Data-parallel: 6 batch elements per core x 8 cores.

Cost-model-driven design:
  - Host marshals inputs to fp16 matmul-ready layouts (casts/transposes of
    weights; x is sent once, its [d,t] transpose is built on device).
  - Every matmul: big operand stationary, small moving -> tiny out free size.
  - Per-step relu tiles [128f, 1024t] spread across DVE / Act / Pool.
  - Batch split in 2 groups of 3; group A's linear gate ops live on DVE,
    group B's on Pool, transcendentals on Act; issue order follows readiness.
  - alpha*xa softmax term accumulated into the racc psum via a second scaled
    w2 column; softmax denominators accumulated per-b on PE.
  - Classifier uses the h16 history and runs every FLUSH steps.
"""

import sys

for _p in ("/opt/trn_rl_repo", "/root/.axon_site/_ro/trn_rl_repo"):
    if _p not in sys.path:
        sys.path.insert(0, _p)

import numpy as np

import concourse.bass as bass
import concourse.bacc as bacc
import concourse.mybir as mybir
import concourse.tile as tile
from concourse import bass_utils, masks

FP32 = mybir.dt.float32
F16 = mybir.dt.float16
AF = mybir.ActivationFunctionType
OP = mybir.AluOpType

B_TOT, T, D, H, F, C, S = 48, 1024, 512, 512, 256, 4367, 22
NCORES = 8
B = B_TOT // NCORES          # 6
ALPHA = 0.01
TC = T // 128                # 8
DC = D // 128                # 4
CPAD = 4480                  # 35*128
CCN = CPAD // 128            # 35
NG = 2
GB = B // NG
FLUSH = 4 if NG == 3 else 6

import os
# relu tile engine plan per group: 2*GB tiles (b-local, fc) -> engine
_PLANS = {
    1: [["v", "s", "v", "g", "v", "v", "v", "s", "v", "v", "g", "v"]],
    2: [["v", "s", "v", "g", "v", "v"],
        ["v", "s", "v", "v", "g", "v"]],
    3: [["v", "s", "v", "g"], ["v", "s", "v", "v"], ["v", "g", "v", "v"]],
}
RELU_PLANS = _PLANS[NG]
_rp = os.environ.get("KRELU", "vsvvvv,vsvvvv")
if _rp:
    RELU_PLANS = [list(p) for p in _rp.split(",")]
KLIN = os.environ.get("KLIN", "vv")
# packed psum column offsets (per group): hw1, racc, srecB, ctx, grz, gin,
# ghn, ssum
O_HW1 = 0
O_RACC = O_HW1 + 2 * GB
O_CTX = O_RACC + GB * TC
O_GRZ = O_CTX + DC * GB
O_GIN = O_GRZ + 8 * GB
O_GHN = O_GIN + 4 * GB
O_CSUM = O_GHN + 4 * GB
O_SRB = O_CSUM + GB * TC
P_COLS = O_SRB + GB


def build(n_steps=S):
    nc = bacc.Bacc("TRN2", target_bir_lowering=False, debug=False,
                   num_devices=NCORES)

    x_d = nc.dram_tensor("x_f16", [128, B, TC, D], F16, kind="ExternalInput").ap()
    w1x_d = nc.dram_tensor("w1x_f16", [128, DC, 2, 128], F16, kind="ExternalInput").ap()
    w1h_d = nc.dram_tensor("w1h_f16", [128, 4, 2, 128], F16, kind="ExternalInput").ap()
    w2_d = nc.dram_tensor("w2x_f16", [128, 2, 2], F16, kind="ExternalInput").ap()
    wiT_d = nc.dram_tensor("wiT_f16", [128, DC, 12, 128], F16, kind="ExternalInput").ap()
    whT_d = nc.dram_tensor("whT_f16", [128, 4, 12, 128], F16, kind="ExternalInput").ap()
    cls_d = nc.dram_tensor("cls_f16", [128, 4, CCN, 128], F16, kind="ExternalInput").ap()
    out_d = nc.dram_tensor("out", [B, S, C], F16, kind="ExternalOutput").ap()

    with tile.TileContext(nc) as tc:
        with tc.tile_pool(name="pers", bufs=1) as pers:
            x_sb = pers.tile([128, B, TC, D], F16)
            w1x_sb = pers.tile([128, DC, 2, 128], F16)
            w1h_sb = pers.tile([128, 4, 2, 128], F16)
            w2_sb = pers.tile([128, 2, 2], F16)
            wiT_sb = pers.tile([128, DC, 12, 128], F16)
            whT_sb = pers.tile([128, 4, 12, 128], F16)
            cls_sb = pers.tile([128, 4, CCN, 128], F16)
            xw1T = pers.tile([128, 2, B, T], F16)    # x@w1x, [fp,(fc,b,t)]
            xeT = pers.tile([128, B, TC], F16)       # exp(alpha*xa)
            ones16 = pers.tile([128, 1], F16)
            ones_row = pers.tile([1, 128], FP32)
            h_sb = pers.tile([128, 4, B], FP32)      # h fp32, [hp,(hc,b)]
            hist = pers.tile([128, 4, S, B], F16)    # h16 history
            ost = pers.tile([128, S, B, CCN], F16)   # out staging c=p*35+cc
            ident = pers.tile([128, 128], F16)

            nc.vector.memset(ones16[:], 1.0)
            nc.vector.memset(ones_row[:], 1.0)
            identf = pers.tile([128, 128], FP32)
            masks.make_identity(nc, identf[:])
            nc.vector.tensor_copy(ident[:], identf[:])

            # ---- prologue: on-device xT, then xw1T = x @ w1x ----
            with tc.tile_pool(name="prsb", bufs=1) as prsb, \
                 tc.tile_pool(name="prps", bufs=1,
                              space=bass.MemorySpace.PSUM) as prps:
                nc.sync.dma_start(w1x_sb[:], w1x_d)
                nc.sync.dma_start(w2_sb[:], w2_d)
                for b in range(B):
                    nc.sync.dma_start(x_sb[:, b], x_d[:, b])
                nc.sync.dma_start(w1h_sb[:], w1h_d)
                nc.sync.dma_start(wiT_sb[:], wiT_d)
                nc.sync.dma_start(whT_sb[:], whT_d)
                nc.sync.dma_start(cls_sb[:], cls_d)

                xt_sb = prsb.tile([128, B, DC, T], F16)

                def pr_transposes(b):
                    for dc in range(DC):
                        tp = prps.tile([128, T], F16, tag="xtp", bufs=3)
                        for tcc in range(TC):
                            nc.tensor.transpose(
                                tp[:, tcc * 128:(tcc + 1) * 128],
                                x_sb[:, b, tcc, dc * 128:(dc + 1) * 128],
                                ident[:])
                        nc.vector.tensor_copy(xt_sb[:, b, dc, 0:512],
                                              tp[:, 0:512])
                        nc.scalar.copy(xt_sb[:, b, dc, 512:1024],
                                       tp[:, 512:1024])

                def pr_mms(b):
                    for fc in range(2):
                        mm = prps.tile([128, T], FP32, tag="xw1", bufs=2)
                        for dc in range(DC):
                            for th in range(2):
                                nc.tensor.matmul(
                                    mm[:, th * 512:(th + 1) * 512],
                                    w1x_sb[:, dc, fc, :],
                                    xt_sb[:, b, dc, th * 512:(th + 1) * 512],
                                    start=(dc == 0), stop=(dc == DC - 1))
                        nc.vector.tensor_copy(xw1T[:, fc, b, 0:512],
                                              mm[:, 0:512])
                        nc.scalar.copy(xw1T[:, fc, b, 512:1024],
                                       mm[:, 512:1024])

                xa = prps.tile([128, B * TC], FP32, tag="xa", bufs=1)

                def pr_xa(b):
                    for tcc in range(TC):
                        for fc in range(2):
                            nc.tensor.matmul(
                                xa[:, b * TC + tcc:b * TC + tcc + 1],
                                xw1T[:, fc, b, tcc * 128:(tcc + 1) * 128],
                                w2_sb[:, fc, 1:2],
                                start=(fc == 0), stop=(fc == 1))

                for b in range(B):
                    pr_transposes(b)
                    if b >= 1:
                        pr_mms(b - 1)
                    if b >= 2:
                        pr_xa(b - 2)
                    if b == 4:
                        # group A's xe factors ready early
                        nc.scalar.activation(
                            xeT[:, 0:GB, :].rearrange("p b t -> p (b t)"),
                            xa[:, 0:GB * TC], AF.Exp, scale=(1.0 - ALPHA))
                pr_mms(B - 1)
                pr_xa(B - 2)
                pr_xa(B - 1)
                nc.scalar.activation(
                    xeT[:, GB:B, :].rearrange("p b t -> p (b t)"),
                    xa[:, GB * TC:B * TC], AF.Exp, scale=(1.0 - ALPHA))

            # ---- step loop ----
            with tc.tile_pool(name="lsb", bufs=1) as lsb, \
                 tc.tile_pool(name="lps", bufs=1,
                              space=bass.MemorySpace.PSUM) as lps:

                _lmap = {"v": nc.vector, "g": nc.gpsimd}
                LIN = [_lmap[c] for c in KLIN][:NG]

                # classifier runs as two batched flushes (steps 0..10 fill
                # PE gaps mid-loop; steps 11..21 run at the end), 5 chunks
                # of 7 cc-columns each.
                CLS_NCH = 5
                CLS_CC = CCN // CLS_NCH            # 7
                _cls_pending = []

                def cls_chunk(s0, s1, ch):
                    ns = s1 - s0
                    cps = lps.tile([128, CLS_CC, ns * B], FP32,
                                   tag="clsf", bufs=2)
                    mov = hist[:, :, s0:s1, :].rearrange("p h s b -> p h (s b)")
                    for cc7 in range(CLS_CC):
                        cc = ch * CLS_CC + cc7
                        for hc in range(4):
                            nc.tensor.matmul(
                                cps[:, cc7, :],
                                cls_sb[:, hc, cc, :],
                                mov[:, hc, :],
                                start=(hc == 0), stop=(hc == 3))
                    _cls_pending.append((cps, s0, s1, ch))

                def cls_copy(eng=None):
                    # gpsimd cannot read PSUM; copies go on Act/DVE only
                    if not _cls_pending:
                        return
                    cps, s0, s1, ch = _cls_pending.pop(0)
                    eng = eng or (nc.scalar if ch % 2 else nc.vector)
                    dst = ost[:, s0:s1, :, ch * CLS_CC:(ch + 1) * CLS_CC]
                    src = cps[:].rearrange("p cc (s b) -> p s b cc", s=s1 - s0)
                    if eng is nc.scalar:
                        eng.copy(dst, src)
                    else:
                        eng.tensor_copy(dst, src)

                def cls_dma(s0, s1):
                    for s in range(s0, s1):
                        nc.sync.dma_start(
                            out_d[:, s, 0:4340].rearrange(
                                "b (p cc) -> p b cc", cc=CCN),
                            ost[0:124, s, :, :])

                P_ = [None] * NG
                csum_ps_ = [None] * NG
                Pnext_ = [None] * NG
                rts_ = [None] * NG
                e2_ = [None] * NG
                ctxT_ = [None] * NG
                trz_ = [None] * NG
                srecB_ = [None] * NG

                def emit_hw1(s, g, P):
                    """next-step attention bias; called from tail_b(s-1)."""
                    b0 = g * GB
                    bsl = slice(b0, b0 + GB)
                    lin = LIN[g]
                    plan = RELU_PLANS[g]
                    hw1 = P[:, O_HW1:O_HW1 + 2 * GB].rearrange(
                        "p (fc b) -> p fc b", fc=2)
                    for fc in range(2):
                        for hc in range(4):
                            nc.tensor.matmul(
                                hw1[:, fc, :],
                                w1h_sb[:, hc, fc, :],
                                hist[:, hc, s - 1, bsl],
                                start=(hc == 0), stop=(hc == 3))
                    hw1a = lsb.tile([128, 2, GB], FP32,
                                    tag=f"hw1a{g}", bufs=2)
                    nc.vector.tensor_copy(hw1a[:], hw1)
                    return hw1a

                hw1a_ = [None] * NG

                def front(s, g):
                    b0 = g * GB
                    lin = LIN[g]
                    plan = RELU_PLANS[g]
                    if Pnext_[g] is None:
                        P0 = lps.tile([128, P_COLS], FP32,
                                      tag=f"P{g}", bufs=2, name=f"P0{g}")
                        Pnext_[g] = (P0, None)
                    P, hw1a = Pnext_[g]
                    hw1a_[g] = hw1a
                    P_[g] = P
                    Pnext_[g] = None
                    hw1 = P[:, O_HW1:O_HW1 + 2 * GB].rearrange(
                        "p (fc b) -> p fc b", fc=2)
                    rts = {}
                    rts_[g] = rts
                    for bl in range(GB):
                        for fc in range(2):
                            rt = lsb.tile([128, T], F16,
                                          tag=f"rt{bl}_{fc}", bufs=3)
                            rts[(bl, fc)] = rt
                            e = plan[bl * 2 + fc]
                            src_ap = xw1T[:, fc, b0 + bl, :]
                            if e == "s":
                                nc.scalar.activation(
                                    rt[:], src_ap, AF.Relu,
                                    bias=(hw1a[:, fc, bl:bl + 1]
                                          if s > 0 else 0.0),
                                    scale=1.0)
                            else:
                                eng = nc.vector if e == "v" else nc.gpsimd
                                if s > 0:
                                    bias = (hw1[:, fc, bl:bl + 1]
                                            if e == "v"
                                            else hw1a[:, fc, bl:bl + 1])
                                    eng.tensor_scalar(
                                        rt[:], src_ap, bias, 0.0,
                                        op0=OP.add, op1=OP.max)
                                else:
                                    eng.tensor_scalar(
                                        rt[:], src_ap, 0.0, 0.0,
                                        op0=OP.add, op1=OP.max)

                def mid_r(s, g):
                    """racc matmuls + exp + e2 mult (PE / Act / LIN)."""
                    b0 = g * GB
                    lin = LIN[g]
                    P = P_[g]
                    racc = P[:, O_RACC:O_RACC + GB * TC].rearrange(
                        "p (b t) -> p b t", b=GB)
                    csum_ps_[g] = P[0:1, O_CSUM:O_CSUM + GB * TC]
                    rts = rts_[g]
                    for bl in range(GB):
                        for tcc in range(TC):
                            tsl = slice(tcc * 128, (tcc + 1) * 128)
                            nc.tensor.matmul(
                                racc[:, bl, tcc:tcc + 1],
                                rts[(bl, 0)][:, tsl],
                                w2_sb[:, 0, 0:1], start=True, stop=False)
                            nc.tensor.matmul(
                                racc[:, bl, tcc:tcc + 1],
                                rts[(bl, 1)][:, tsl],
                                w2_sb[:, 1, 0:1], start=False, stop=True)
                    e2f = lsb.tile([128, GB, TC], F16, tag=f"e2f{g}", bufs=2)
                    nc.scalar.activation(
                        e2f[:].rearrange("p b t -> p (b t)"),
                        racc.rearrange("p b t -> p (b t)"),
                        AF.Exp, scale=(1.0 - ALPHA))
                    e2 = lsb.tile([128, GB, TC, 1], F16, tag=f"e2{g}", bufs=2)
                    e2_[g] = e2
                    lin.tensor_tensor(e2[:, :, :, 0], e2f[:],
                                      xeT[:, b0:b0 + GB, :], op=OP.mult)

                def mid_c(s, g):
                    """csum + ctx matmuls (PE, queued after other group's
                    GRU stream so the exp/e2 latency is hidden)."""
                    b0 = g * GB
                    P = P_[g]
                    ctx = P[:, O_CTX:O_CTX + DC * GB].rearrange(
                        "p (dc b) -> p dc b", dc=DC)
                    e2 = e2_[g]
                    nc.tensor.matmul(
                        csum_ps_[g], ones16[:],
                        e2[:, :, :, 0].rearrange("p b t -> p (b t)"),
                        start=True, stop=True)
                    for bl in range(GB):
                        for dc in range(DC):
                            for tcc in range(TC):
                                nc.tensor.matmul(
                                    ctx[:, dc, bl:bl + 1],
                                    x_sb[:, b0 + bl, tcc,
                                         dc * 128:(dc + 1) * 128],
                                    e2[:, bl, tcc, :],
                                    start=(tcc == 0), stop=(tcc == TC - 1))

                def mid_s(s, g):
                    """softmax denominators + ctxT scale (DVE / PE)."""
                    P = P_[g]
                    ctx = P[:, O_CTX:O_CTX + DC * GB].rearrange(
                        "p (dc b) -> p dc b", dc=DC)
                    srec = lsb.tile([1, GB], FP32, tag=f"srec{g}", bufs=2)
                    nc.vector.tensor_reduce(
                        srec[:], csum_ps_[g].rearrange("p (b t) -> p b t",
                                                       b=GB),
                        axis=mybir.AxisListType.X, op=OP.add)
                    nc.vector.reciprocal(srec[:], srec[:])
                    srecB = P[:, O_SRB:O_SRB + GB]
                    nc.tensor.matmul(srecB, ones_row[:], srec[:],
                                     start=True, stop=True)
                    srecB_[g] = srecB
                    ctxT = lsb.tile([128, DC, GB], F16, tag=f"ctxT{g}", bufs=2)
                    ctxT_[g] = ctxT
                    for bl in range(GB):
                        nc.vector.tensor_scalar(
                            ctxT[:, :, bl], ctx[:, :, bl],
                            srecB[:, bl:bl + 1], 0.0, op0=OP.mult)

                def tail_a(s, g):
                    b0 = g * GB
                    bsl = slice(b0, b0 + GB)
                    P = P_[g]
                    grz = P[:, O_GRZ:O_GRZ + 8 * GB].rearrange(
                        "p (m b) -> p m b", m=8)
                    gin = P[:, O_GIN:O_GIN + 4 * GB].rearrange(
                        "p (m b) -> p m b", m=4)
                    ghn = P[:, O_GHN:O_GHN + 4 * GB].rearrange(
                        "p (m b) -> p m b", m=4)
                    ctxT = ctxT_[g]
                    for m in range(12):
                        dst = grz[:, m, :] if m < 8 else gin[:, m - 8, :]
                        last_src = (s == 0 or m >= 8)
                        for dc in range(DC):
                            nc.tensor.matmul(
                                dst, wiT_sb[:, dc, m, :], ctxT[:, dc, :],
                                start=(dc == 0),
                                stop=(dc == DC - 1 and last_src))
                    if s > 0:
                        for m in range(12):
                            dst = grz[:, m, :] if m < 8 else ghn[:, m - 8, :]
                            for hc in range(4):
                                nc.tensor.matmul(
                                    dst, whT_sb[:, hc, m, :],
                                    hist[:, hc, s - 1, bsl],
                                    start=(m >= 8 and hc == 0),
                                    stop=(hc == 3))
                    t_rz = lsb.tile([128, 8, GB], FP32, tag=f"trz{g}", bufs=2)
                    nc.scalar.activation(
                        t_rz[:].rearrange("p m b -> p (m b)"),
                        grz.rearrange("p m b -> p (m b)"),
                        AF.Tanh, scale=0.5)
                    trz_[g] = t_rz

                def tail_b(s, g):
                    b0 = g * GB
                    bsl = slice(b0, b0 + GB)
                    lin = LIN[g]
                    P = P_[g]
                    gin = P[:, O_GIN:O_GIN + 4 * GB].rearrange(
                        "p (m b) -> p m b", m=4)
                    ghn = P[:, O_GHN:O_GHN + 4 * GB].rearrange(
                        "p (m b) -> p m b", m=4)
                    t_rz = trz_[g]
                    n_t = lsb.tile([128, 4, GB], FP32, tag=f"nt{g}", bufs=2)
                    if s > 0:
                        g2 = lsb.tile([128, 4, GB], FP32, tag=f"g2{g}", bufs=2)
                        nc.vector.scalar_tensor_tensor(
                            g2[:], t_rz[:, 0:4, :], 1.0, ghn,
                            op0=OP.add, op1=OP.mult)
                        g4 = lsb.tile([128, 4, GB], FP32, tag=f"g4{g}", bufs=2)
                        nc.vector.scalar_tensor_tensor(
                            g4[:], g2[:], 0.5, gin,
                            op0=OP.mult, op1=OP.add)
                        nc.scalar.activation(
                            n_t[:].rearrange("p m b -> p (m b)"),
                            g4[:].rearrange("p m b -> p (m b)"),
                            AF.Tanh, scale=1.0)
                    else:
                        nc.scalar.activation(
                            n_t[:].rearrange("p m b -> p (m b)"),
                            gin.rearrange("p m b -> p (m b)"),
                            AF.Tanh, scale=1.0)
                    qz = lsb.tile([128, 4, GB], FP32, tag=f"qz{g}", bufs=2)
                    nc.scalar.activation(
                        qz[:].rearrange("p m b -> p (m b)"),
                        t_rz[:, 4:8, :].rearrange("p m b -> p (m b)"),
                        AF.Copy, bias=0.5, scale=-0.5)    # 1-z
                    if s > 0:
                        zh2 = lsb.tile([128, 4, GB], FP32,
                                       tag=f"zh2{g}", bufs=2)
                        lin.scalar_tensor_tensor(
                            zh2[:], t_rz[:, 4:8, :], 1.0,
                            hist[:, :, s - 1, bsl],
                            op0=OP.add, op1=OP.mult)      # 2*z*h
                        nq = lsb.tile([128, 4, GB], FP32, tag=f"nq{g}", bufs=2)
                        lin.tensor_tensor(nq[:], n_t[:], qz[:], op=OP.mult)
                        lin.scalar_tensor_tensor(
                            hist[:, :, s, bsl], zh2[:], 0.5, nq[:],
                            op0=OP.mult, op1=OP.add)
                    else:
                        lin.tensor_tensor(
                            hist[:, :, s, bsl], n_t[:], qz[:], op=OP.mult)
                    # prepare next step: allocate P(s+1) and compute hw1
                    if s + 1 < n_steps:
                        Pn = lps.tile([128, P_COLS], FP32, tag=f"P{g}", bufs=2)
                        hw1a = emit_hw1(s + 1, g, Pn)
                        Pnext_[g] = (Pn, hw1a)

                # flush A covers steps 0..SPLIT-1, chunks fill PE gaps in
                # steps SPLIT+1..SPLIT+5; flush B (steps SPLIT..21) runs
                # after the loop.
                SPLIT = 11
                tasks = [(s, g) for s in range(n_steps) for g in range(NG)]
                for k in range(len(tasks) + 2):
                    if 0 <= k - 1 < len(tasks):
                        mid_r(*tasks[k - 1])
                    if 0 <= k - 2 < len(tasks):
                        tail_a(*tasks[k - 2])
                    if 0 <= k - 1 < len(tasks):
                        mid_c(*tasks[k - 1])
                    if 0 <= k - 2 < len(tasks):
                        ts_, tg_ = tasks[k - 2]
                        if tg_ == NG - 1 and SPLIT <= ts_ < SPLIT + CLS_NCH:
                            cls_chunk(0, SPLIT, ts_ - SPLIT)
                        tail_b(ts_, tg_)
                    if 0 <= k - 1 < len(tasks):
                        mid_s(*tasks[k - 1])
                    if k < len(tasks):
                        front(*tasks[k])
                    if 0 <= k - 2 < len(tasks):
                        ts_, tg_ = tasks[k - 2]
                        if tg_ == NG - 1:
                            cls_copy()
                            if ts_ == SPLIT + CLS_NCH:
                                cls_dma(0, SPLIT)
                # drain: flush B (steps SPLIT..S-1) + remaining DMAs
                for ch in range(CLS_NCH):
                    cls_chunk(SPLIT, S, ch)
                    cls_copy()
                cls_dma(SPLIT, S)
                for b in range(B):
                    nc.sync.dma_start(
                        out_d[b:b + 1, :, 4340:C],
                        ost[124:125, :, b, 0:C - 4340])

    nc.compile()
    return nc


_NC = None


def _get_nc():
    global _NC
    if _NC is None:
        _NC = build()
    return _NC


def _marshal(x_core, w1, w2, wi, wh, cls_w):
    """Host-side input staging: casts / transposes / padding only."""
    m = {}
    f16 = np.float16
    x = np.ascontiguousarray(x_core).astype(f16)           # [B,T,D]
    m["x_f16"] = np.ascontiguousarray(
        x.reshape(B, TC, 128, D).transpose(2, 0, 1, 3))
    w1f = np.asarray(w1, np.float32).astype(f16)           # [D+H, F]
    m["w1x_f16"] = np.ascontiguousarray(
        w1f[:D].reshape(DC, 128, 2, 128).transpose(1, 0, 2, 3))
    m["w1h_f16"] = np.ascontiguousarray(
        w1f[D:].reshape(4, 128, 2, 128).transpose(1, 0, 2, 3))
    w2v = np.asarray(w2, np.float32)[:, 0].reshape(2, 128)  # [fc, p]
    w2x = np.stack([w2v, w2v * (ALPHA / (1.0 - ALPHA))], axis=-1)
    m["w2x_f16"] = np.ascontiguousarray(w2x.transpose(1, 0, 2).astype(f16))
    for name, w in (("wiT_f16", wi), ("whT_f16", wh)):
        wT = np.asarray(w, np.float32).T.astype(f16)       # [D, 3H]
        m[name] = np.ascontiguousarray(
            wT.reshape(4, 128, 12, 128).transpose(1, 0, 2, 3))
    cpad = np.zeros((CPAD, H), np.float32)
    cpad[:C] = np.asarray(cls_w, np.float32)
    ct = cpad.T.astype(f16)                                # [H, CPAD]
    m["cls_f16"] = np.ascontiguousarray(
        ct.reshape(4, 128, 128, CCN).transpose(1, 0, 3, 2))
    return m


def run(inputs, trace=False, **kw):
    nc = _get_nc()
    full = {k: np.asarray(v, dtype=np.float32) for k, v in inputs.items()}
    in_maps = []
    for c in range(NCORES):
        in_maps.append(_marshal(
            full["x"][c * B:(c + 1) * B], full["attn_w1"], full["attn_w2"],
            full["gru_wi"], full["gru_wh"], full["cls_w"]))
    res = bass_utils.run_bass_kernel_spmd(
        nc, in_maps, core_ids=list(range(NCORES)), trace=trace, **kw)
    out = np.concatenate(
        [res.results[c]["out"] for c in range(NCORES)], axis=0
    ).astype(np.float32)
    return out, res


def kernel(**inputs) -> np.ndarray:
    out, _ = run(inputs, trace=False)
    return out

